# revision 11
# baseline (speedup 1.0000x reference)
"""Trainium2 Bass kernel for a 2-layer GRU decoder step (B=32768, H=256).

Sharding: pure data parallel — batch split across 8 NeuronCores, weights
replicated. On-chip layout is feature-major ([H, B_core]); the host does the
transposes so the device pipeline (linear -> GRU1 -> GRU2) needs no on-chip
transposes at all: each matmul's PSUM output [M=features, N=batch] feeds the
next stage directly.

Matmuls run in fp16 (full PE rate: 2-byte moving operand streams 2 cols/cycle,
fast weight loads); PSUM accumulation and the gate math stay fp32, so the
end-to-end error vs the fp32 reference is ~6e-4. Inputs arrive packed in one
fp16 block per chunk (single large DMA), outputs leave packed in one fp32
block per chunk.
"""

import sys

if "/opt/trn_rl_repo" not in sys.path:
    sys.path.insert(0, "/opt/trn_rl_repo")

import numpy as np

import concourse.bacc as bacc
import concourse.mybir as mybir
from concourse import bass, tile
from concourse.bass_utils import run_bass_kernel_spmd

N_CORES = 8
B = 32768
H = 256
BC = B // N_CORES  # batch rows per core

F32 = mybir.dt.float32
F16 = mybir.dt.float16
AF = mybir.ActivationFunctionType
OP = mybir.AluOpType

# Tunables
BT = 512  # batch-tile (matmul moving dim; one fp32 PSUM bank)
CW = 512  # DMA chunk width (batch cols per streamed chunk)
GATE_BUFS = 10

KT_CAT = 4  # 512 concat features / 128
KT_H = 2  # 256 features / 128
G_IN = KT_CAT + 2 * KT_H  # packed input groups: cat(4) + h1(2) + h2(2)
G_OUT = 3 * KT_H  # packed output groups: xout(2) + h0(2) + h1(2)
NCH = BC // CW

_last_results = None  # stashed BassKernelResults (for the test harness)
_built = {}


def _build():
    key = (BT, CW, GATE_BUFS)
    if key in _built:
        return _built[key]

    nc = bacc.Bacc("TRN2", target_bir_lowering=False, debug=False)

    in_d = nc.dram_tensor("in_p", [NCH, 128, G_IN, CW], F16, kind="ExternalInput")
    wlin_d = nc.dram_tensor("wlin_t", [2 * H, H], F16, kind="ExternalInput")
    wih1_d = nc.dram_tensor("wih1_t", [H, 3 * H], F16, kind="ExternalInput")
    whh1_d = nc.dram_tensor("whh1_t", [H, 3 * H], F16, kind="ExternalInput")
    wih2_d = nc.dram_tensor("wih2_t", [H, 3 * H], F16, kind="ExternalInput")
    whh2_d = nc.dram_tensor("whh2_t", [H, 3 * H], F16, kind="ExternalInput")
    bias_d = nc.dram_tensor("bias_p", [128, 16], F32, kind="ExternalInput")
    out_d = nc.dram_tensor("out_p", [NCH, 128, G_OUT, CW], F16, kind="ExternalOutput")

    def fm(ap, kt):  # DRAM [kt*128, N] -> feature-major [p, kt, N]
        return ap.ap().rearrange("(kt p) b -> p kt b", p=128)

    with tile.TileContext(nc) as tc:
        with (
            tc.tile_pool(name="wpool", bufs=1) as wp,
            tc.tile_pool(name="inpool", bufs=3) as ip,
            tc.tile_pool(name="xpool", bufs=3) as xp,
            tc.tile_pool(name="opool", bufs=2) as op_,
            tc.tile_pool(name="gates", bufs=GATE_BUFS) as gp,
            tc.tile_pool(name="psum", bufs=8, space="PSUM") as pp,
        ):
            wlin_t = wp.tile([128, KT_CAT, H], F16, tag="wlin")
            wih1_t = wp.tile([128, KT_H, 3 * H], F16, tag="wih1")
            whh1_t = wp.tile([128, KT_H, 3 * H], F16, tag="whh1")
            wih2_t = wp.tile([128, KT_H, 3 * H], F16, tag="wih2")
            whh2_t = wp.tile([128, KT_H, 3 * H], F16, tag="whh2")
            bias_t = wp.tile([128, 16], F32, tag="bias")
            bias = {
                "brz1": bias_t[:, 0:4],
                "bin1": bias_t[:, 4:6],
                "bhn1": bias_t[:, 6:8],
                "brz2": bias_t[:, 8:12],
                "bin2": bias_t[:, 12:14],
                "bhn2": bias_t[:, 14:16],
            }

            def mm_group(ps, w, rhs, w_ms, kts, start, stop):
                nkt = len(kts)
                for i, kt in enumerate(kts):
                    nc.tensor.matmul(
                        ps[:],
                        w[:, kt, w_ms],
                        rhs[:, kt, :],
                        start=(start and i == 0),
                        stop=(stop and i == nkt - 1),
                    )

            def gru_mm_gh(h_in, whh):
                """x-independent half: gh contributions (rz accum start) + hn."""
                rz_ps, hn_ps = [], []
                for mt in range(4):
                    ps = pp.tile([128, BT], F32, tag="ps")
                    mm_group(ps, whh, h_in, bass.ts(mt, 128), range(KT_H), True, False)
                    rz_ps.append(ps)
                for ft in range(2):
                    ps = pp.tile([128, BT], F32, tag="ps")
                    mm_group(ps, whh, h_in, bass.ts(4 + ft, 128), range(KT_H), True, True)
                    hn_ps.append(ps)
                return rz_ps, hn_ps

            def gru_mm_gi(st, x_in, wih):
                rz_ps, _ = st["gh"]
                for mt in range(4):
                    mm_group(rz_ps[mt], wih, x_in, bass.ts(mt, 128), range(KT_H), False, True)
                in_ps = []
                for ft in range(2):
                    ps = pp.tile([128, BT], F32, tag="ps")
                    mm_group(ps, wih, x_in, bass.ts(4 + ft, 128), range(KT_H), True, True)
                    in_ps.append(ps)
                st["in"] = in_ps

            def gru_elem(st, x_in, h_in, brz, bin_, bhn, h_out, x_out, filler=None):
                """Gate math, ft=0/1 chains interleaved to hide engine hops."""
                rz_ps, hn_ps = st["gh"]
                in_ps = st["in"]
                r, z, t, t2, n, d, u = {}, {}, {}, {}, {}, {}, {}
                for ft in range(2):
                    r[ft] = gp.tile([128, BT], F32, tag="r", name="r")
                    nc.scalar.activation(
                        r[ft][:], rz_ps[ft][:], AF.Sigmoid, bias=brz[:, ft : ft + 1]
                    )
                    z[ft] = gp.tile([128, BT], F16, tag="z", name="z")
                    nc.scalar.activation(
                        z[ft][:], rz_ps[2 + ft][:], AF.Sigmoid,
                        bias=brz[:, 2 + ft : 3 + ft],
                    )
                if filler is not None:
                    filler()  # ACT work that fits while DVE runs t/t2
                for ft in range(2):
                    t[ft] = gp.tile([128, BT], F32, tag="t", name="t")
                    nc.vector.scalar_tensor_tensor(
                        t[ft][:], hn_ps[ft][:], bhn[:, ft : ft + 1], r[ft][:],
                        OP.add, OP.mult,
                    )
                for ft in range(2):
                    t2[ft] = gp.tile([128, BT], F32, tag="t2", name="t2")
                    nc.vector.tensor_tensor(t2[ft][:], in_ps[ft][:], t[ft][:], OP.add)
                for ft in range(2):
                    n[ft] = gp.tile([128, BT], F16, tag="n", name="n")
                    nc.scalar.activation(
                        n[ft][:], t2[ft][:], AF.Tanh, bias=bin_[:, ft : ft + 1]
                    )
                ho_t, ho_g, ho_s = h_out
                xo_t, xo_g, xo_s = x_out
                for ft in range(2):
                    d[ft] = gp.tile([128, BT], F16, tag="d", name="d")
                    nc.vector.tensor_tensor(d[ft][:], h_in[:, ft, :], n[ft][:], OP.subtract)
                    u[ft] = gp.tile([128, BT], F16, tag="u", name="u")
                    nc.vector.tensor_tensor(u[ft][:], z[ft][:], d[ft][:], OP.mult)
                    hp = ho_t[:, ho_g + ft, ho_s]
                    nc.vector.tensor_tensor(hp, n[ft][:], u[ft][:], OP.add)
                    nc.vector.tensor_tensor(
                        xo_t[:, xo_g + ft, xo_s], x_in[:, ft, :], hp, OP.add
                    )

            # --- software pipeline over b-tiles (CW == BT: one b-tile per chunk) ---
            S = [dict() for _ in range(NCH)]
            bs = bass.ts(0, BT)  # CW == BT

            def dma_in(j):
                S[j]["in"] = ip.tile([128, G_IN, CW], F16, tag="in", name="in_t")
                nc.sync.dma_start(S[j]["in"][:], in_d.ap()[j])

            def views(j):
                in_t = S[j]["in"]
                return (
                    in_t[:, 0:KT_CAT, bs],
                    in_t[:, KT_CAT : KT_CAT + KT_H, bs],
                    in_t[:, KT_CAT + KT_H : G_IN, bs],
                )

            def lin_mm(j):
                cat_v, _, _ = views(j)
                S[j]["lin_ps"] = []
                for mt in range(KT_H):
                    ps = pp.tile([128, BT], F32, tag="ps")
                    mm_group(ps, wlin_t, cat_v, bass.ts(mt, 128), range(KT_CAT), True, True)
                    S[j]["lin_ps"].append(ps)

            def lin_copy(j):
                S[j]["xlin"] = xp.tile([128, KT_H, CW], F16, tag="xlin", name="xlin_t")
                for mt in range(KT_H):
                    nc.scalar.copy(S[j]["xlin"][:, mt, bs], S[j]["lin_ps"][mt][:])

            # PE warmup: dummy matmuls during the initial DMA wait pull the
            # HAM clock gate to 8/8 before the first real matmul issues
            warm_w = wp.tile([128, 128], F16, tag="warm")
            nc.gpsimd.memset(warm_w[:], 0.0)
            warm_ps = pp.tile([128, 128], F32, tag="ps")
            for _ in range(40):
                nc.tensor.matmul(
                    warm_ps[:], warm_w[:], warm_w[:], start=True, stop=True
                )
            nc.sync.dma_start(wlin_t[:], fm(wlin_d, KT_CAT))
            dma_in(0)
            nc.sync.dma_start(wih1_t[:], fm(wih1_d, KT_H))
            nc.sync.dma_start(whh1_t[:], fm(whh1_d, KT_H))
            nc.sync.dma_start(bias_t[:], bias_d.ap())
            dma_in(1)
            nc.sync.dma_start(wih2_t[:], fm(wih2_d, KT_H))
            nc.sync.dma_start(whh2_t[:], fm(whh2_d, KT_H))
            lin_mm(0)
            lin_copy(0)
            g1 = {}
            _, h1_v0, _ = views(0)
            g1["gh"] = gru_mm_gh(h1_v0, whh1_t)
            gru_mm_gi(g1, S[0]["xlin"][:, :, bs], wih1_t)
            S[0]["g1"] = g1

            for j in range(NCH):
                if j + 2 < NCH:
                    dma_in(j + 2)
                if j + 1 < NCH:
                    lin_mm(j + 1)
                cat_v, h1_v, h2_v = views(j)
                out_t = op_.tile([128, G_OUT, CW], F16, tag="out")
                x1_t = xp.tile([128, KT_H, CW], F16, tag="x1")
                g2 = {}
                g2["gh"] = gru_mm_gh(h2_v, whh2_t)

                def filler(jn=j + 1):
                    if jn < NCH:
                        lin_copy(jn)

                gru_elem(
                    S[j]["g1"], S[j]["xlin"][:, :, bs], h1_v,
                    bias["brz1"], bias["bin1"], bias["bhn1"],
                    (out_t, KT_H, bs), (x1_t, 0, bs), filler,
                )
                gru_mm_gi(g2, x1_t[:, :, bs], wih2_t)
                if j + 1 < NCH:
                    g1n = {}
                    _, h1_vn, _ = views(j + 1)
                    g1n["gh"] = gru_mm_gh(h1_vn, whh1_t)
                    gru_mm_gi(g1n, S[j + 1]["xlin"][:, :, bs], wih1_t)
                    S[j + 1]["g1"] = g1n
                gru_elem(
                    g2, x1_t[:, :, bs], h2_v,
                    bias["brz2"], bias["bin2"], bias["bhn2"],
                    (out_t, 2 * KT_H, bs), (out_t, 0, bs),
                )
                nc.sync.dma_start(out_d.ap()[j], out_t[:])

    nc.compile()
    _built[key] = nc
    return nc


def _bias_fm(b):  # [k*128] -> [128, k] feature-major
    return np.ascontiguousarray(b.reshape(-1, 128).T)


def kernel(
    attn_out,
    attn_rnn_hidden,
    dec_rnn_hiddens,
    W_lin,
    gru1_Wih,
    gru1_Whh,
    gru1_bih,
    gru1_bhh,
    gru2_Wih,
    gru2_Whh,
    gru2_bih,
    gru2_bhh,
):
    global _last_results
    f = np.float32

    # packed fp16 feature-major input: groups = cat(4) + h1(2) + h2(2),
    # each group 128 features x B batch
    gin = np.empty((G_IN, 128, B), dtype=np.float16)
    gin[0:2] = np.asarray(attn_rnn_hidden).T.astype(np.float16).reshape(2, 128, B)
    gin[2:4] = np.asarray(attn_out).T.astype(np.float16).reshape(2, 128, B)
    gin[4:6] = np.asarray(dec_rnn_hiddens[0]).T.astype(np.float16).reshape(2, 128, B)
    gin[6:8] = np.asarray(dec_rnn_hiddens[1]).T.astype(np.float16).reshape(2, 128, B)

    shared = {
        "wlin_t": np.ascontiguousarray(W_lin.T).astype(np.float16),
        "wih1_t": np.ascontiguousarray(gru1_Wih.T).astype(np.float16),
        "whh1_t": np.ascontiguousarray(gru1_Whh.T).astype(np.float16),
        "wih2_t": np.ascontiguousarray(gru2_Wih.T).astype(np.float16),
        "whh2_t": np.ascontiguousarray(gru2_Whh.T).astype(np.float16),
        "bias_p": np.concatenate(
            [
                _bias_fm((gru1_bih + gru1_bhh)[: 2 * H].astype(f)),
                _bias_fm(gru1_bih[2 * H :].astype(f)),
                _bias_fm(gru1_bhh[2 * H :].astype(f)),
                _bias_fm((gru2_bih + gru2_bhh)[: 2 * H].astype(f)),
                _bias_fm(gru2_bih[2 * H :].astype(f)),
                _bias_fm(gru2_bhh[2 * H :].astype(f)),
            ],
            axis=1,
        ),
    }
    in_maps = []
    for c in range(N_CORES):
        s = slice(c * BC, (c + 1) * BC)
        m = dict(shared)
        # [G_IN, 128, BC] -> [NCH, 128, G_IN, CW]
        blk = gin[:, :, s].reshape(G_IN, 128, NCH, CW)
        m["in_p"] = np.ascontiguousarray(blk.transpose(2, 1, 0, 3))
        in_maps.append(m)

    nc = _build()
    res = run_bass_kernel_spmd(nc, in_maps, core_ids=list(range(N_CORES)))
    _last_results = res

    xT = np.empty((2, 128, B), dtype=f)
    h0T = np.empty((2, 128, B), dtype=f)
    h1T = np.empty((2, 128, B), dtype=f)
    for c in range(N_CORES):
        s = slice(c * BC, (c + 1) * BC)
        # [NCH, 128, G_OUT, CW] -> [G_OUT, 128, BC]
        blk = (
            res.results[c]["out_p"]
            .astype(f)
            .transpose(2, 1, 0, 3)
            .reshape(G_OUT, 128, BC)
        )
        xT[:, :, s] = blk[0:2]
        h0T[:, :, s] = blk[2:4]
        h1T[:, :, s] = blk[4:6]
    x = np.ascontiguousarray(xT.reshape(H, B).T)
    hiddens = np.stack(
        [
            np.ascontiguousarray(h0T.reshape(H, B).T),
            np.ascontiguousarray(h1T.reshape(H, B).T),
        ],
        axis=0,
    )
    return x, hiddens


# revision 12
# speedup vs baseline: 1.0380x; 1.0380x over previous
"""Trainium2 Bass kernel for a 2-layer GRU decoder step (B=32768, H=256).

Sharding: pure data parallel — batch split across 8 NeuronCores, weights
replicated. On-chip layout is feature-major ([H, B_core]); the host does the
transposes so the device pipeline (linear -> GRU1 -> GRU2) needs no on-chip
transposes at all: each matmul's PSUM output [M=features, N=batch] feeds the
next stage directly.

Matmuls run in fp16 (full PE rate: 2-byte moving operand streams 2 cols/cycle,
fast weight loads); PSUM accumulation and the gate math stay fp32, so the
end-to-end error vs the fp32 reference is ~6e-4. Inputs arrive packed in one
fp16 block per chunk (single large DMA), outputs leave packed in one fp32
block per chunk.
"""

import sys

if "/opt/trn_rl_repo" not in sys.path:
    sys.path.insert(0, "/opt/trn_rl_repo")

import numpy as np

import concourse.bacc as bacc
import concourse.mybir as mybir
from concourse import bass, tile
from concourse.bass_utils import run_bass_kernel_spmd

N_CORES = 8
B = 32768
H = 256
BC = B // N_CORES  # batch rows per core

F32 = mybir.dt.float32
F16 = mybir.dt.float16
AF = mybir.ActivationFunctionType
OP = mybir.AluOpType

# Tunables
BT = 512  # batch-tile (matmul moving dim; one fp32 PSUM bank)
CW = 512  # DMA chunk width (batch cols per streamed chunk)
GATE_BUFS = 10

KT_CAT = 4  # 512 concat features / 128
KT_H = 2  # 256 features / 128
G_IN = KT_CAT + 2 * KT_H  # packed input groups: cat(4) + h1(2) + h2(2)
G_OUT = 3 * KT_H  # packed output groups: xout(2) + h0(2) + h1(2)
NCH = BC // CW

_last_results = None  # stashed BassKernelResults (for the test harness)
_built = {}


def _build():
    key = (BT, CW, GATE_BUFS)
    if key in _built:
        return _built[key]

    nc = bacc.Bacc("TRN2", target_bir_lowering=False, debug=False)

    in_d = nc.dram_tensor("in_p", [NCH, 128, G_IN, CW], F16, kind="ExternalInput")
    wlin_d = nc.dram_tensor("wlin_t", [2 * H, H], F16, kind="ExternalInput")
    wih1_d = nc.dram_tensor("wih1_t", [H, 3 * H], F16, kind="ExternalInput")
    whh1_d = nc.dram_tensor("whh1_t", [H, 3 * H], F16, kind="ExternalInput")
    wih2_d = nc.dram_tensor("wih2_t", [H, 3 * H], F16, kind="ExternalInput")
    whh2_d = nc.dram_tensor("whh2_t", [H, 3 * H], F16, kind="ExternalInput")
    bias_d = nc.dram_tensor("bias_p", [128, 16], F32, kind="ExternalInput")
    out_d = nc.dram_tensor("out_p", [NCH, 128, G_OUT, CW], F16, kind="ExternalOutput")

    def fm(ap, kt):  # DRAM [kt*128, N] -> feature-major [p, kt, N]
        return ap.ap().rearrange("(kt p) b -> p kt b", p=128)

    with tile.TileContext(nc) as tc:
        with (
            tc.tile_pool(name="wpool", bufs=1) as wp,
            tc.tile_pool(name="inpool", bufs=3) as ip,
            tc.tile_pool(name="xpool", bufs=3) as xp,
            tc.tile_pool(name="opool", bufs=2) as op_,
            tc.tile_pool(name="gates", bufs=GATE_BUFS) as gp,
            tc.tile_pool(name="psum", bufs=8, space="PSUM") as pp,
        ):
            wlin_t = wp.tile([128, KT_CAT, H], F16, tag="wlin")
            wih1_t = wp.tile([128, KT_H, 3 * H], F16, tag="wih1")
            whh1_t = wp.tile([128, KT_H, 3 * H], F16, tag="whh1")
            wih2_t = wp.tile([128, KT_H, 3 * H], F16, tag="wih2")
            whh2_t = wp.tile([128, KT_H, 3 * H], F16, tag="whh2")
            bias_t = wp.tile([128, 16], F32, tag="bias")
            bias = {
                "brz1": bias_t[:, 0:4],
                "bin1": bias_t[:, 4:6],
                "bhn1": bias_t[:, 6:8],
                "brz2": bias_t[:, 8:12],
                "bin2": bias_t[:, 12:14],
                "bhn2": bias_t[:, 14:16],
            }

            def mm_group(ps, w, rhs, w_ms, kts, start, stop):
                nkt = len(kts)
                for i, kt in enumerate(kts):
                    nc.tensor.matmul(
                        ps[:],
                        w[:, kt, w_ms],
                        rhs[:, kt, :],
                        start=(start and i == 0),
                        stop=(stop and i == nkt - 1),
                    )

            def gru_mm_gh_rz(h_in, whh):
                """x-independent: gh contributions into the r/z banks."""
                rz_ps = []
                for mt in range(4):
                    ps = pp.tile([128, BT], F32, tag="ps")
                    mm_group(ps, whh, h_in, bass.ts(mt, 128), range(KT_H), True, False)
                    rz_ps.append(ps)
                return rz_ps

            def gru_mm_gh_hn(h_in, whh):
                hn_ps = []
                for ft in range(2):
                    ps = pp.tile([128, BT], F32, tag="ps")
                    mm_group(ps, whh, h_in, bass.ts(4 + ft, 128), range(KT_H), True, True)
                    hn_ps.append(ps)
                return hn_ps

            def gru_mm_gh(h_in, whh):
                return gru_mm_gh_rz(h_in, whh), gru_mm_gh_hn(h_in, whh)

            def gru_mm_gi(st, x_in, wih):
                rz_ps, _ = st["gh"]
                for mt in range(4):
                    mm_group(rz_ps[mt], wih, x_in, bass.ts(mt, 128), range(KT_H), False, True)
                in_ps = []
                for ft in range(2):
                    ps = pp.tile([128, BT], F32, tag="ps")
                    mm_group(ps, wih, x_in, bass.ts(4 + ft, 128), range(KT_H), True, True)
                    in_ps.append(ps)
                st["in"] = in_ps

            def gru_elem(st, x_in, h_in, brz, bin_, bhn, h_out, x_out, filler=None):
                """Gate math, ft=0/1 chains interleaved to hide engine hops."""
                rz_ps, hn_ps = st["gh"]
                in_ps = st["in"]
                r, z, t, t2, n, d, u = {}, {}, {}, {}, {}, {}, {}
                for ft in range(2):
                    r[ft] = gp.tile([128, BT], F32, tag="r", name="r")
                    nc.scalar.activation(
                        r[ft][:], rz_ps[ft][:], AF.Sigmoid, bias=brz[:, ft : ft + 1]
                    )
                    if ft == 0:
                        z[0] = gp.tile([128, BT], F16, tag="z", name="z")
                        nc.scalar.activation(
                            z[0][:], rz_ps[2][:], AF.Sigmoid, bias=brz[:, 2:3]
                        )
                if filler is not None:
                    filler()  # ACT work that fits while DVE runs t/t2
                z[1] = gp.tile([128, BT], F16, tag="z", name="z")
                nc.scalar.activation(z[1][:], rz_ps[3][:], AF.Sigmoid, bias=brz[:, 3:4])
                for ft in range(2):
                    t[ft] = gp.tile([128, BT], F32, tag="t", name="t")
                    nc.vector.scalar_tensor_tensor(
                        t[ft][:], hn_ps[ft][:], bhn[:, ft : ft + 1], r[ft][:],
                        OP.add, OP.mult,
                    )
                for ft in range(2):
                    t2[ft] = gp.tile([128, BT], F32, tag="t2", name="t2")
                    nc.vector.tensor_tensor(t2[ft][:], in_ps[ft][:], t[ft][:], OP.add)
                for ft in range(2):
                    n[ft] = gp.tile([128, BT], F16, tag="n", name="n")
                    nc.scalar.activation(
                        n[ft][:], t2[ft][:], AF.Tanh, bias=bin_[:, ft : ft + 1]
                    )
                ho_t, ho_g, ho_s = h_out
                xo_t, xo_g, xo_s = x_out
                for ft in range(2):
                    d[ft] = gp.tile([128, BT], F16, tag="d", name="d")
                    nc.vector.tensor_tensor(d[ft][:], h_in[:, ft, :], n[ft][:], OP.subtract)
                    u[ft] = gp.tile([128, BT], F16, tag="u", name="u")
                    nc.vector.tensor_tensor(u[ft][:], z[ft][:], d[ft][:], OP.mult)
                    hp = ho_t[:, ho_g + ft, ho_s]
                    nc.vector.tensor_tensor(hp, n[ft][:], u[ft][:], OP.add)
                    nc.vector.tensor_tensor(
                        xo_t[:, xo_g + ft, xo_s], x_in[:, ft, :], hp, OP.add
                    )

            # --- software pipeline over b-tiles (CW == BT: one b-tile per chunk) ---
            S = [dict() for _ in range(NCH)]
            bs = bass.ts(0, BT)  # CW == BT

            def dma_in(j):
                S[j]["in"] = ip.tile([128, G_IN, CW], F16, tag="in", name="in_t")
                nc.sync.dma_start(S[j]["in"][:], in_d.ap()[j])

            def views(j):
                in_t = S[j]["in"]
                return (
                    in_t[:, 0:KT_CAT, bs],
                    in_t[:, KT_CAT : KT_CAT + KT_H, bs],
                    in_t[:, KT_CAT + KT_H : G_IN, bs],
                )

            def lin_mm(j):
                cat_v, _, _ = views(j)
                S[j]["lin_ps"] = []
                for mt in range(KT_H):
                    ps = pp.tile([128, BT], F32, tag="ps")
                    mm_group(ps, wlin_t, cat_v, bass.ts(mt, 128), range(KT_CAT), True, True)
                    S[j]["lin_ps"].append(ps)

            def lin_copy(j):
                S[j]["xlin"] = xp.tile([128, KT_H, CW], F16, tag="xlin", name="xlin_t")
                for mt in range(KT_H):
                    nc.scalar.copy(S[j]["xlin"][:, mt, bs], S[j]["lin_ps"][mt][:])

            # PE warmup: dummy matmuls during the initial DMA wait pull the
            # HAM clock gate to 8/8 before the first real matmul issues
            warm_w = wp.tile([128, 128], F16, tag="warm")
            nc.gpsimd.memset(warm_w[:], 0.0)
            warm_ps = pp.tile([128, 128], F32, tag="ps")
            for _ in range(40):
                nc.tensor.matmul(
                    warm_ps[:], warm_w[:], warm_w[:], start=True, stop=True
                )
            nc.sync.dma_start(wlin_t[:], fm(wlin_d, KT_CAT))
            dma_in(0)
            nc.sync.dma_start(wih1_t[:], fm(wih1_d, KT_H))
            nc.sync.dma_start(whh1_t[:], fm(whh1_d, KT_H))
            nc.sync.dma_start(bias_t[:], bias_d.ap())
            dma_in(1)
            nc.sync.dma_start(wih2_t[:], fm(wih2_d, KT_H))
            nc.sync.dma_start(whh2_t[:], fm(whh2_d, KT_H))
            lin_mm(0)
            lin_copy(0)
            g1 = {}
            _, h1_v0, _ = views(0)
            g1["gh"] = gru_mm_gh(h1_v0, whh1_t)
            gru_mm_gi(g1, S[0]["xlin"][:, :, bs], wih1_t)
            S[0]["g1"] = g1

            for j in range(NCH):
                if j + 2 < NCH:
                    dma_in(j + 2)
                if j + 1 < NCH:
                    lin_mm(j + 1)
                cat_v, h1_v, h2_v = views(j)
                out_t = op_.tile([128, G_OUT, CW], F16, tag="out")
                x1_t = xp.tile([128, KT_H, CW], F16, tag="x1")
                g2 = {}
                g2["gh"] = gru_mm_gh(h2_v, whh2_t)

                def filler(jn=j + 1):
                    if jn < NCH:
                        lin_copy(jn)

                gru_elem(
                    S[j]["g1"], S[j]["xlin"][:, :, bs], h1_v,
                    bias["brz1"], bias["bin1"], bias["bhn1"],
                    (out_t, KT_H, bs), (x1_t, 0, bs), filler,
                )
                g1n_hn = None
                if j + 1 < NCH:
                    _, h1_vn, _ = views(j + 1)
                    g1n_hn = gru_mm_gh_hn(h1_vn, whh1_t)  # PE fill before x1 wait
                gru_mm_gi(g2, x1_t[:, :, bs], wih2_t)
                if j + 1 < NCH:
                    g1n = {}
                    g1n["gh"] = (gru_mm_gh_rz(h1_vn, whh1_t), g1n_hn)
                    gru_mm_gi(g1n, S[j + 1]["xlin"][:, :, bs], wih1_t)
                    S[j + 1]["g1"] = g1n
                gru_elem(
                    g2, x1_t[:, :, bs], h2_v,
                    bias["brz2"], bias["bin2"], bias["bhn2"],
                    (out_t, 2 * KT_H, bs), (out_t, 0, bs),
                )
                nc.sync.dma_start(out_d.ap()[j], out_t[:])

    nc.compile()
    _built[key] = nc
    return nc


def _bias_fm(b):  # [k*128] -> [128, k] feature-major
    return np.ascontiguousarray(b.reshape(-1, 128).T)


def kernel(
    attn_out,
    attn_rnn_hidden,
    dec_rnn_hiddens,
    W_lin,
    gru1_Wih,
    gru1_Whh,
    gru1_bih,
    gru1_bhh,
    gru2_Wih,
    gru2_Whh,
    gru2_bih,
    gru2_bhh,
):
    global _last_results
    f = np.float32

    # packed fp16 feature-major input: groups = cat(4) + h1(2) + h2(2),
    # each group 128 features x B batch
    gin = np.empty((G_IN, 128, B), dtype=np.float16)
    gin[0:2] = np.asarray(attn_rnn_hidden).T.astype(np.float16).reshape(2, 128, B)
    gin[2:4] = np.asarray(attn_out).T.astype(np.float16).reshape(2, 128, B)
    gin[4:6] = np.asarray(dec_rnn_hiddens[0]).T.astype(np.float16).reshape(2, 128, B)
    gin[6:8] = np.asarray(dec_rnn_hiddens[1]).T.astype(np.float16).reshape(2, 128, B)

    shared = {
        "wlin_t": np.ascontiguousarray(W_lin.T).astype(np.float16),
        "wih1_t": np.ascontiguousarray(gru1_Wih.T).astype(np.float16),
        "whh1_t": np.ascontiguousarray(gru1_Whh.T).astype(np.float16),
        "wih2_t": np.ascontiguousarray(gru2_Wih.T).astype(np.float16),
        "whh2_t": np.ascontiguousarray(gru2_Whh.T).astype(np.float16),
        "bias_p": np.concatenate(
            [
                _bias_fm((gru1_bih + gru1_bhh)[: 2 * H].astype(f)),
                _bias_fm(gru1_bih[2 * H :].astype(f)),
                _bias_fm(gru1_bhh[2 * H :].astype(f)),
                _bias_fm((gru2_bih + gru2_bhh)[: 2 * H].astype(f)),
                _bias_fm(gru2_bih[2 * H :].astype(f)),
                _bias_fm(gru2_bhh[2 * H :].astype(f)),
            ],
            axis=1,
        ),
    }
    in_maps = []
    for c in range(N_CORES):
        s = slice(c * BC, (c + 1) * BC)
        m = dict(shared)
        # [G_IN, 128, BC] -> [NCH, 128, G_IN, CW]
        blk = gin[:, :, s].reshape(G_IN, 128, NCH, CW)
        m["in_p"] = np.ascontiguousarray(blk.transpose(2, 1, 0, 3))
        in_maps.append(m)

    nc = _build()
    res = run_bass_kernel_spmd(nc, in_maps, core_ids=list(range(N_CORES)))
    _last_results = res

    xT = np.empty((2, 128, B), dtype=f)
    h0T = np.empty((2, 128, B), dtype=f)
    h1T = np.empty((2, 128, B), dtype=f)
    for c in range(N_CORES):
        s = slice(c * BC, (c + 1) * BC)
        # [NCH, 128, G_OUT, CW] -> [G_OUT, 128, BC]
        blk = (
            res.results[c]["out_p"]
            .astype(f)
            .transpose(2, 1, 0, 3)
            .reshape(G_OUT, 128, BC)
        )
        xT[:, :, s] = blk[0:2]
        h0T[:, :, s] = blk[2:4]
        h1T[:, :, s] = blk[4:6]
    x = np.ascontiguousarray(xT.reshape(H, B).T)
    hiddens = np.stack(
        [
            np.ascontiguousarray(h0T.reshape(H, B).T),
            np.ascontiguousarray(h1T.reshape(H, B).T),
        ],
        axis=0,
    )
    return x, hiddens
